# revision 31
# baseline (speedup 1.0000x reference)
"""Trainium2 Bass kernel for IR-Net style binarized 3x3 conv + BN + Hardtanh.

Reference computation:
  bw = sign(standardize(weight)) * sw   (sw = per-cout power-of-2 scale)
  ba = sign(x)
  y  = clip(conv3x3(ba, bw) * bn_scale + bn_bias, -1, 1)

Both matmul operands are exactly +-1, which is exactly representable in
fp8e4m3, so the conv runs as fp8 DoubleRow matmuls on the TensorEngine
with zero numerical error (fp32 PSUM accumulation of integers <= 2304).
All binarization is host-side prep: x ships as fp8 +-1 sign planes that
are already zero-padded and cin-chunk-interleaved, so activations DMA
straight into their SBUF matmul layout — no on-device binarize, border
memsets, or staging.  sw and the BN affine fold into one per-channel
scale/bias applied in the epilogue on VectorE.

Distribution: pure data parallel, 32 images -> 4 per NeuronCore, full
weights replicated, no collectives.

Layout: per-image zero-padded activation planes in SBUF, fp8, with the
two cin-128-chunks byte-interleaved as the DoubleRow k-subtile dim.
Rows are 57 wide (56 data + 1 shared zero column: col 0 is row r's left
pad AND row r-1's right pad), so each of the 9 conv taps is a contiguous
shifted window of the flattened plane and only 1 of every 57 output
columns is garbage.  The conv is 9 accumulated DoubleRow matmuls
([128,2,128] @ [128,2,456], K=256) per 8-row output tile.

Scheduling: dependency waits on DMA-written tiles coalesce to the
NEWEST DMA issued on that hardware ring at schedule time, so every
dma_start is placed in program order immediately before its first
consumer, split into just-in-time pieces (img0 in 3 row-bands, co=0
weights in 2 tap-groups).  A burst of dummy matmuls on a zeroed scratch
tile warms the PE HAM clock gate before the real stream starts.
"""

import numpy as np

import concourse.bass as bass
import concourse.bacc as bacc
import concourse.mybir as mybir
import concourse.tile as tile
from concourse.bass_utils import run_bass_kernel_spmd

B, CIN, COUT, H, W = 32, 256, 256, 56, 56
NCORES = 8
BPC = B // NCORES            # images per core
HP = H + 2                   # padded rows
RW = W + 1                   # row width: 56 data + 1 shared zero col
IMG = HP * RW                # 3306
GUARD = 64                   # front zero guard (shifted windows stay in bounds)
XT = 3376                    # GUARD + IMG + tail guard(6); %16==0 for DoubleRow
RB = 8                       # output rows per tile
NBLK = H // RB               # 7
NCI = CIN // 128             # 2 cin chunks = DoubleRow k-subtiles
NCO = COUT // 128            # 2 cout chunks
KTAPS = 9
BN_EPS = 1e-5

# img0 row-band split points (tile elem index): rows 0-10 / 11-26 / rest
S1 = GUARD + 11 * RW
S2 = GUARD + 27 * RW

F32 = mybir.dt.float32
FP8 = mybir.dt.float8e4
FP8NP = mybir.dt.np(FP8)

_CACHE: dict = {}


def _build_nc() -> bass.Bass:
    nc = bacc.Bacc("TRN2", target_bir_lowering=False, debug=False, num_devices=NCORES)
    xin8 = nc.declare_dram_parameter("xin8", [BPC, 128, XT * NCI], FP8, isOutput=False)
    wts = nc.declare_dram_parameter(
        "wts", [128, KTAPS * NCO * NCI * 128], FP8, isOutput=False
    )
    sb = nc.declare_dram_parameter("sb", [128, 2 * NCO], F32, isOutput=False)
    yout = nc.declare_dram_parameter("yout", [BPC, COUT, H, W], F32, isOutput=True)

    with tile.TileContext(nc) as tc:
        with (
            tc.tile_pool(name="const", bufs=1) as cpool,
            tc.tile_pool(name="psum", bufs=8, space=bass.MemorySpace.PSUM) as ppool,
            tc.tile_pool(name="ot", bufs=8) as otpool,
            tc.tile_pool(name="oc", bufs=12) as ocpool,
        ):
            # weights: [p, (co, k, j, m)]
            w_sb = cpool.tile([128, KTAPS * NCO * NCI * 128], FP8, tag="w")
            sb_sb = cpool.tile([128, 2 * NCO], F32, tag="sb")
            WTAP = NCI * 128          # 256 B per tap per partition
            WCO = KTAPS * WTAP        # one cout chunk
            w4 = w_sb.rearrange("p (co k j m) -> p k co j m", k=KTAPS, co=NCO, j=NCI)

            # Zero scratch for PE warmup operands (dedicated tile so warmup
            # reads never overlap anything written later).
            wz = cpool.tile([128, 256], FP8, tag="wz")
            nc.vector.memset(wz[:], 0.0)

            # Padded binarized activation planes, one tile per image;
            # entirely DMA-written (borders ship as zeros from the host).
            xp = {}
            for img in range(BPC):
                t = cpool.tile([128, XT, NCI], FP8, tag=f"xp{img}", name=f"xp{img}")
                xp[img] = t

            def ld_piece(img, a, b, eng):
                return eng.dma_start(
                    xp[img][:, a:b, :], xin8[img, :, a * NCI : b * NCI]
                )

            # Startup DMAs.  Any instruction that consumes DMA-written data
            # waits on the NEWEST DMA the scheduler issued before it (global
            # watermark), and the scheduler front-loads every ungated DMA.
            # So the ungated set is exactly what the first matmul needs —
            # img0 rows 0-10, scale/bias, co=0 weights (~455 KB) — and every
            # other transfer is semaphore-gated behind the first matmul.
            sc_chain = [nc.scalar.dma_start(sb_sb[:], sb[:])]
            sc_chain.append(
                nc.scalar.dma_start(w_sb[:, 0 : 3 * WTAP], wts[:, 0 : 3 * WTAP])
            )
            sq_chain = [ld_piece(0, 0, S1, nc.sync)]
            gq_chain = []

            # PE warmup: dummy DoubleRow matmuls on the zeroed scratch tile,
            # with operand access patterns identical in structure to the
            # real ones (the dual-fp8 LDWEIGHTS path is picky).  They only
            # depend on the scratch memset, so they start ~2us before the
            # first real matmul and flip the HAM clock gate to 8/8 by the
            # time the stream begins.
            wm_ps = ppool.tile([128, 120], F32, tag="ps")
            wm_lhs = wz[:, 0:256].rearrange("p (j m) -> p j m", j=2)
            wm_rhs = wz[:, 0:240].rearrange("p (x j) -> p j x", j=2)
            wms = []
            for _ in range(20):
                wms.append(nc.tensor.matmul(
                    wm_ps[:],
                    wm_lhs,
                    wm_rhs,
                    start=True,
                    stop=True,
                    perf_mode=mybir.MatmulPerfMode.DoubleRow,
                ))

            def gate_dma(dma, trigger):
                # real semaphore gate on an early trigger (so the transfer
                # starts promptly) plus a schedule-order-only edge after the
                # first matmul (so mm0's global DMA watermark excludes it)
                tile.add_dep_helper(dma.ins, trigger.ins, sync=True,
                                    reason="JIT DMA trigger")
                tile.add_dep_helper(dma.ins, mm0.ins, sync=False,
                                    reason="keep out of mm0 watermark")

            mm0 = None
            for img in range(BPC):
                for co in range(NCO):
                    if img == 0 and co == 1:
                        # co=1 weights and the bulk image loads
                        wc1 = nc.scalar.dma_start(
                            w_sb[:, WCO : 2 * WCO], wts[:, WCO : 2 * WCO]
                        )
                        gate_dma(wc1, wms[8])
                        sc_chain.append(wc1)
                        for im2 in range(1, BPC):
                            dma = ld_piece(im2, 0, XT, nc.gpsimd)
                            gate_dma(dma, mm0)
                            gq_chain.append(dma)
                    s_ap = sb_sb[:, co : co + 1]
                    b_ap = sb_sb[:, NCO + co : NCO + co + 1]
                    # (start padded row, rows) per output tile; the final
                    # tiles of the kernel are split so the last epilogue +
                    # store chain after the last matmul is as short as
                    # possible, fanned out across otherwise-idle queues.
                    blocks = [(1 + b * RB, RB, nc.sync) for b in range(NBLK)]
                    if img == BPC - 1 and co == NCO - 1:
                        blocks = blocks[:-1] + [
                            (49, 4, nc.sync),
                            (53, 2, nc.gpsimd),
                            (55, 2, nc.scalar),
                        ]
                    for bi, (y0p, rb, oq) in enumerate(blocks):
                        if img == 0 and co == 0 and bi == 1:
                            # img0 rows 11-57 (two pieces), triggered off
                            # early warmup matmuls so they land before
                            # blocks 1 and 3 need them
                            for (a, b2), trig in (((S1, S2), wms[4]),
                                                  ((S2, XT), wms[6])):
                                dma = ld_piece(0, a, b2, nc.gpsimd)
                                gate_dma(dma, trig)
                                gq_chain.append(dma)
                        nt = rb * RW
                        ps = ppool.tile([128, nt], F32, tag="ps")
                        for k in range(KTAPS):
                            ky, kx = divmod(k, 3)
                            s0 = GUARD + (y0p + ky - 1) * RW + kx
                            rhs = xp[img][:, s0 : s0 + nt, :].rearrange(
                                "p x j -> p j x"
                            )
                            mm = nc.tensor.matmul(
                                ps[:],
                                w4[:, k, co],
                                rhs,
                                start=(k == 0),
                                stop=(k == KTAPS - 1),
                                perf_mode=mybir.MatmulPerfMode.DoubleRow,
                            )
                            if mm0 is None:
                                mm0 = mm
                            if img == 0 and co == 0 and bi == 0 and k in (2, 5):
                                # next tap-group weights: triggered early,
                                # scheduled after this tap so earlier taps'
                                # waits don't cover them
                                a = (k + 1) * WTAP
                                b2 = (k + 4) * WTAP
                                eng = nc.scalar if k == 2 else nc.sync
                                wnext = eng.dma_start(
                                    w_sb[:, a:b2], wts[:, a:b2]
                                )
                                tile.add_dep_helper(
                                    wnext.ins, wms[1 if k == 2 else 3].ins,
                                    sync=True, reason="tap group trigger",
                                )
                                tile.add_dep_helper(
                                    wnext.ins, mm.ins, sync=False,
                                    reason="keep out of earlier tap watermark",
                                )
                                (sc_chain if k == 2 else sq_chain).append(wnext)
                        ot = otpool.tile([128, nt], F32, tag="ot")
                        nc.vector.tensor_scalar(
                            ot[:],
                            ps[:],
                            s_ap,
                            b_ap,
                            op0=mybir.AluOpType.mult,
                            op1=mybir.AluOpType.add,
                        )
                        # clip + compact away the garbage col per row, so
                        # both sides of the output DMA are fully contiguous
                        oc = ocpool.tile([128, rb * W], F32, tag="oc")
                        nc.vector.tensor_scalar(
                            oc[:],
                            ot.rearrange("p (r c) -> p r c", c=RW)[:, :, 0:W],
                            -1.0,
                            1.0,
                            op0=mybir.AluOpType.max,
                            op1=mybir.AluOpType.min,
                        )
                        oq.dma_start(
                            yout[img, co * 128 : (co + 1) * 128, y0p - 1 : y0p - 1 + rb, :],
                            oc[:],
                        )
            # pin issue order per ring (ring packet order = issue order)
            for ch in (sc_chain, sq_chain, gq_chain):
                for a, b in zip(ch, ch[1:]):
                    tile.add_dep_helper(
                        b.ins, a.ins, sync=False, reason="startup DMA issue order"
                    )
    nc.finalize()
    return nc


def get_nc() -> bass.Bass:
    if "nc" not in _CACHE:
        _CACHE["nc"] = _build_nc()
    return _CACHE["nc"]


def _host_prep(weight, gamma, beta, running_mean, running_var):
    """Binarize standardized weights, fold sw + BN into scale/bias."""
    wf = weight.reshape(COUT, -1).astype(np.float64)
    n = wf.shape[1]
    mean = wf.mean(axis=1, keepdims=True)
    d = wf - mean
    sgn = np.where(d >= 0, 1.0, -1.0)
    std = np.sqrt((d * d).sum(axis=1, keepdims=True) / (n - 1))
    bw = d / std
    sw = np.exp2(np.round(np.log2(np.abs(bw).mean(axis=1))))  # [COUT]
    inv = gamma.astype(np.float64) / np.sqrt(running_var.astype(np.float64) + BN_EPS)
    scale = (sw * inv).astype(np.float32)
    bias = (beta.astype(np.float64) - running_mean.astype(np.float64) * inv).astype(
        np.float32
    )

    # wts[p, (co, k, j, m)] = sgn[co*128+m, (j*128+p)*9 + k]
    w6 = sgn.reshape(NCO, 128, NCI, 128, KTAPS)  # [co, m, j, p, k]
    wts = (
        np.ascontiguousarray(np.transpose(w6, (3, 0, 4, 2, 1)))  # p co k j m
        .reshape(128, KTAPS * NCO * NCI * 128)
        .astype(FP8NP)
    )
    # sb[m, co] = scale chunk, sb[m, NCO+co] = bias chunk
    sbarr = np.concatenate(
        [scale.reshape(NCO, 128).T, bias.reshape(NCO, 128).T], axis=1
    ).astype(np.float32)
    sbarr = np.ascontiguousarray(sbarr)
    return wts, sbarr


def _host_signs(x):
    """fp8 +-1 sign planes, zero-padded 58x57 rows, cin-chunk interleaved.

    out[b, p, t, j] = fp8(sign(x[b, j*128+p, r-1, c-1])) at t = GUARD+r*57+c
    for the interior, 0 elsewhere (pads/guards), matching torch.sign
    (sign(0) = 0).
    """
    xv = x.reshape(B, NCI, 128, H, W)
    xs = ((xv < 0).astype(np.uint8) * 0x80) | ((xv != 0).astype(np.uint8) * 0x38)
    out = np.zeros((B, 128, XT, NCI), np.uint8)
    interior = out[:, :, GUARD : GUARD + IMG, :].reshape(B, 128, HP, RW, NCI)
    interior[:, :, 1 : H + 1, 1 : W + 1, :] = xs.transpose(0, 2, 3, 4, 1)
    return out.reshape(B, 128, XT * NCI).view(FP8NP)


def run(x, weight, gamma, beta, running_mean, running_var, trace=False, **tkw):
    x = np.asarray(x, dtype=np.float32)
    wts, sbarr = _host_prep(
        np.asarray(weight, dtype=np.float32),
        np.asarray(gamma, dtype=np.float32),
        np.asarray(beta, dtype=np.float32),
        np.asarray(running_mean, dtype=np.float32),
        np.asarray(running_var, dtype=np.float32),
    )
    x8 = _host_signs(x)
    in_maps = [
        {
            "xin8": x8[c * BPC : (c + 1) * BPC],
            "wts": wts,
            "sb": sbarr,
        }
        for c in range(NCORES)
    ]
    nc = get_nc()
    res = run_bass_kernel_spmd(nc, in_maps, list(range(NCORES)), trace=trace, **tkw)
    y = np.concatenate([r["yout"] for r in res.results], axis=0)
    return y.astype(np.float32, copy=False), res


def kernel(x, weight, gamma, beta, running_mean, running_var):
    y, _ = run(x, weight, gamma, beta, running_mean, running_var)
    return y


# revision 35
# speedup vs baseline: 1.0003x; 1.0003x over previous
"""Trainium2 Bass kernel for IR-Net style binarized 3x3 conv + BN + Hardtanh.

Reference computation:
  bw = sign(standardize(weight)) * sw   (sw = per-cout power-of-2 scale)
  ba = sign(x)
  y  = clip(conv3x3(ba, bw) * bn_scale + bn_bias, -1, 1)

Both matmul operands are exactly +-1, which is exactly representable in
fp8e4m3, so the conv runs as fp8 DoubleRow matmuls on the TensorEngine
with zero numerical error (fp32 PSUM accumulation of integers <= 2304).
All binarization is host-side prep: x ships as fp8 +-1 sign planes that
are already zero-padded and cin-chunk-interleaved, so activations DMA
straight into their SBUF matmul layout — no on-device binarize, border
memsets, or staging.  sw and the BN affine fold into one per-channel
scale/bias applied in the epilogue on VectorE.

Distribution: pure data parallel, 32 images -> 4 per NeuronCore, full
weights replicated, no collectives.

Layout: per-image zero-padded activation planes in SBUF, fp8, with the
two cin-128-chunks byte-interleaved as the DoubleRow k-subtile dim.
Rows are 57 wide (56 data + 1 shared zero column: col 0 is row r's left
pad AND row r-1's right pad), so each of the 9 conv taps is a contiguous
shifted window of the flattened plane and only 1 of every 57 output
columns is garbage.  The conv is 9 accumulated DoubleRow matmuls
([128,2,128] @ [128,2,456], K=256) per 8-row output tile.

Scheduling: dependency waits on DMA-written tiles coalesce to the
NEWEST DMA issued on that hardware ring at schedule time, so every
dma_start is placed in program order immediately before its first
consumer, split into just-in-time pieces (img0 in 3 row-bands, co=0
weights in 2 tap-groups).  A burst of dummy matmuls on a zeroed scratch
tile warms the PE HAM clock gate before the real stream starts.
"""

import numpy as np

import concourse.bass as bass
import concourse.bacc as bacc
import concourse.mybir as mybir
import concourse.tile as tile
from concourse.bass_utils import run_bass_kernel_spmd

B, CIN, COUT, H, W = 32, 256, 256, 56, 56
NCORES = 8
BPC = B // NCORES            # images per core
HP = H + 2                   # padded rows
RW = W + 1                   # row width: 56 data + 1 shared zero col
IMG = HP * RW                # 3306
GUARD = 64                   # front zero guard (shifted windows stay in bounds)
XT = 3376                    # GUARD + IMG + tail guard(6); %16==0 for DoubleRow
RB = 8                       # output rows per tile
NBLK = H // RB               # 7
NCI = CIN // 128             # 2 cin chunks = DoubleRow k-subtiles
NCO = COUT // 128            # 2 cout chunks
KTAPS = 9
BN_EPS = 1e-5

# img0 row-band split points (tile elem index): rows 0-10 / 11-26 / rest
S1 = GUARD + 11 * RW
S2 = GUARD + 27 * RW

F32 = mybir.dt.float32
FP8 = mybir.dt.float8e4
FP8NP = mybir.dt.np(FP8)

_CACHE: dict = {}


def _build_nc() -> bass.Bass:
    nc = bacc.Bacc("TRN2", target_bir_lowering=False, debug=False, num_devices=NCORES)
    xin8 = nc.declare_dram_parameter("xin8", [BPC, 128, XT * NCI], FP8, isOutput=False)
    wts = nc.declare_dram_parameter(
        "wts", [128, KTAPS * NCO * NCI * 128], FP8, isOutput=False
    )
    sb = nc.declare_dram_parameter("sb", [128, 2 * NCO], F32, isOutput=False)
    yout = nc.declare_dram_parameter("yout", [BPC, COUT, H, W], F32, isOutput=True)

    with tile.TileContext(nc) as tc:
        with (
            tc.tile_pool(name="const", bufs=1) as cpool,
            tc.tile_pool(name="psum", bufs=8, space=bass.MemorySpace.PSUM) as ppool,
            tc.tile_pool(name="ot", bufs=8) as otpool,
            tc.tile_pool(name="oc", bufs=12) as ocpool,
        ):
            # weights: [p, (co, k, j, m)]
            w_sb = cpool.tile([128, KTAPS * NCO * NCI * 128], FP8, tag="w")
            sb_sb = cpool.tile([128, 2 * NCO], F32, tag="sb")
            WTAP = NCI * 128          # 256 B per tap per partition
            WCO = KTAPS * WTAP        # one cout chunk
            w4 = w_sb.rearrange("p (co k j m) -> p k co j m", k=KTAPS, co=NCO, j=NCI)

            # Zero scratch for PE warmup operands (dedicated tile so warmup
            # reads never overlap anything written later).
            wz = cpool.tile([128, 512], FP8, tag="wz")
            nc.vector.memset(wz[:], 0.0)

            # Padded binarized activation planes, one tile per image;
            # entirely DMA-written (borders ship as zeros from the host).
            xp = {}
            for img in range(BPC):
                t = cpool.tile([128, XT, NCI], FP8, tag=f"xp{img}", name=f"xp{img}")
                xp[img] = t

            def ld_piece(img, a, b, eng):
                return eng.dma_start(
                    xp[img][:, a:b, :], xin8[img, :, a * NCI : b * NCI]
                )

            # Startup DMAs.  Any instruction that consumes DMA-written data
            # waits on the NEWEST DMA the scheduler issued before it (global
            # watermark), and the scheduler front-loads every ungated DMA.
            # So the ungated set is exactly what the first matmul needs —
            # img0 rows 0-10, scale/bias, co=0 weights (~455 KB) — and every
            # other transfer is semaphore-gated behind the first matmul.
            sc_chain = [nc.scalar.dma_start(sb_sb[:], sb[:])]
            sc_chain.append(nc.scalar.dma_start(w_sb[:, 0:WCO], wts[:, 0:WCO]))
            sq_chain = [ld_piece(0, 0, S1, nc.sync)]
            gq_chain = []

            # PE warmup: dummy DoubleRow matmuls on the zeroed scratch tile,
            # with operand access patterns identical in structure to the
            # real ones (the dual-fp8 LDWEIGHTS path is picky).  They only
            # depend on the scratch memset, so they start ~2us before the
            # first real matmul and flip the HAM clock gate to 8/8 by the
            # time the stream begins.
            # Normal-mode fp8 (FWL hides the weight load), N=500: each MM
            # covers ~210ns, so 22 of them keep the PE continuously busy
            # from ~+0.5us until the real weights land (~+4.5us) — HAM
            # flips to 8/8 mid-warmup and the real stream starts at full
            # clock with no idle gap to re-throttle it.
            wm_ps = ppool.tile([128, 500], F32, tag="ps")
            wms = []
            for _ in range(22):
                wms.append(nc.tensor.matmul(
                    wm_ps[:],
                    wz[:, 0:128],
                    wz[:, 0:500],
                    start=True,
                    stop=True,
                ))

            def gate_dma(dma, trigger):
                # real semaphore gate on an early trigger (so the transfer
                # starts promptly) plus a schedule-order-only edge after the
                # first matmul (so mm0's global DMA watermark excludes it)
                tile.add_dep_helper(dma.ins, trigger.ins, sync=True,
                                    reason="JIT DMA trigger")
                tile.add_dep_helper(dma.ins, mm0.ins, sync=False,
                                    reason="keep out of mm0 watermark")

            mm0 = None
            for img in range(BPC):
                for co in range(NCO):
                    if img == 0 and co == 1:
                        # co=1 weights and the bulk image loads
                        wc1 = nc.scalar.dma_start(
                            w_sb[:, WCO : 2 * WCO], wts[:, WCO : 2 * WCO]
                        )
                        gate_dma(wc1, wms[8])
                        sc_chain.append(wc1)
                        for im2 in range(1, BPC):
                            dma = ld_piece(im2, 0, XT, nc.gpsimd)
                            gate_dma(dma, mm0)
                            gq_chain.append(dma)
                    s_ap = sb_sb[:, co : co + 1]
                    b_ap = sb_sb[:, NCO + co : NCO + co + 1]
                    # (start padded row, rows) per output tile; the final
                    # tiles of the kernel are split so the last epilogue +
                    # store chain after the last matmul is as short as
                    # possible, fanned out across otherwise-idle queues.
                    blocks = [(1 + b * RB, RB, nc.sync) for b in range(NBLK)]
                    if img == BPC - 1 and co == NCO - 1:
                        blocks = blocks[:-1] + [
                            (49, 4, nc.sync),
                            (53, 2, nc.gpsimd),
                            (55, 2, nc.scalar),
                        ]
                    for bi, (y0p, rb, oq) in enumerate(blocks):
                        if img == 0 and co == 0 and bi == 1:
                            # img0 rows 11-57 (two pieces), triggered off
                            # early warmup matmuls so they land before
                            # blocks 1 and 3 need them
                            for (a, b2), trig in (((S1, S2), wms[4]),
                                                  ((S2, XT), wms[6])):
                                dma = ld_piece(0, a, b2, nc.gpsimd)
                                gate_dma(dma, trig)
                                gq_chain.append(dma)
                        nt = rb * RW
                        ps = ppool.tile([128, nt], F32, tag="ps")
                        for k in range(KTAPS):
                            ky, kx = divmod(k, 3)
                            s0 = GUARD + (y0p + ky - 1) * RW + kx
                            rhs = xp[img][:, s0 : s0 + nt, :].rearrange(
                                "p x j -> p j x"
                            )
                            mm = nc.tensor.matmul(
                                ps[:],
                                w4[:, k, co],
                                rhs,
                                start=(k == 0),
                                stop=(k == KTAPS - 1),
                                perf_mode=mybir.MatmulPerfMode.DoubleRow,
                            )
                            if mm0 is None:
                                mm0 = mm

                        ot = otpool.tile([128, nt], F32, tag="ot")
                        nc.vector.tensor_scalar(
                            ot[:],
                            ps[:],
                            s_ap,
                            b_ap,
                            op0=mybir.AluOpType.mult,
                            op1=mybir.AluOpType.add,
                        )
                        # clip + compact away the garbage col per row, so
                        # both sides of the output DMA are fully contiguous
                        oc = ocpool.tile([128, rb * W], F32, tag="oc")
                        nc.vector.tensor_scalar(
                            oc[:],
                            ot.rearrange("p (r c) -> p r c", c=RW)[:, :, 0:W],
                            -1.0,
                            1.0,
                            op0=mybir.AluOpType.max,
                            op1=mybir.AluOpType.min,
                        )
                        oq.dma_start(
                            yout[img, co * 128 : (co + 1) * 128, y0p - 1 : y0p - 1 + rb, :],
                            oc[:],
                        )
            # pin issue order per ring (ring packet order = issue order)
            for ch in (sc_chain, sq_chain, gq_chain):
                for a, b in zip(ch, ch[1:]):
                    tile.add_dep_helper(
                        b.ins, a.ins, sync=False, reason="startup DMA issue order"
                    )
    nc.finalize()
    return nc


def get_nc() -> bass.Bass:
    if "nc" not in _CACHE:
        _CACHE["nc"] = _build_nc()
    return _CACHE["nc"]


def _host_prep(weight, gamma, beta, running_mean, running_var):
    """Binarize standardized weights, fold sw + BN into scale/bias."""
    wf = weight.reshape(COUT, -1).astype(np.float64)
    n = wf.shape[1]
    mean = wf.mean(axis=1, keepdims=True)
    d = wf - mean
    sgn = np.where(d >= 0, 1.0, -1.0)
    std = np.sqrt((d * d).sum(axis=1, keepdims=True) / (n - 1))
    bw = d / std
    sw = np.exp2(np.round(np.log2(np.abs(bw).mean(axis=1))))  # [COUT]
    inv = gamma.astype(np.float64) / np.sqrt(running_var.astype(np.float64) + BN_EPS)
    scale = (sw * inv).astype(np.float32)
    bias = (beta.astype(np.float64) - running_mean.astype(np.float64) * inv).astype(
        np.float32
    )

    # wts[p, (co, k, j, m)] = sgn[co*128+m, (j*128+p)*9 + k]
    w6 = sgn.reshape(NCO, 128, NCI, 128, KTAPS)  # [co, m, j, p, k]
    wts = (
        np.ascontiguousarray(np.transpose(w6, (3, 0, 4, 2, 1)))  # p co k j m
        .reshape(128, KTAPS * NCO * NCI * 128)
        .astype(FP8NP)
    )
    # sb[m, co] = scale chunk, sb[m, NCO+co] = bias chunk
    sbarr = np.concatenate(
        [scale.reshape(NCO, 128).T, bias.reshape(NCO, 128).T], axis=1
    ).astype(np.float32)
    sbarr = np.ascontiguousarray(sbarr)
    return wts, sbarr


def _host_signs(x):
    """fp8 +-1 sign planes, zero-padded 58x57 rows, cin-chunk interleaved.

    out[b, p, t, j] = fp8(sign(x[b, j*128+p, r-1, c-1])) at t = GUARD+r*57+c
    for the interior, 0 elsewhere (pads/guards), matching torch.sign
    (sign(0) = 0).
    """
    xv = x.reshape(B, NCI, 128, H, W)
    xs = ((xv < 0).astype(np.uint8) * 0x80) | ((xv != 0).astype(np.uint8) * 0x38)
    out = np.zeros((B, 128, XT, NCI), np.uint8)
    interior = out[:, :, GUARD : GUARD + IMG, :].reshape(B, 128, HP, RW, NCI)
    interior[:, :, 1 : H + 1, 1 : W + 1, :] = xs.transpose(0, 2, 3, 4, 1)
    return out.reshape(B, 128, XT * NCI).view(FP8NP)


def run(x, weight, gamma, beta, running_mean, running_var, trace=False, **tkw):
    x = np.asarray(x, dtype=np.float32)
    wts, sbarr = _host_prep(
        np.asarray(weight, dtype=np.float32),
        np.asarray(gamma, dtype=np.float32),
        np.asarray(beta, dtype=np.float32),
        np.asarray(running_mean, dtype=np.float32),
        np.asarray(running_var, dtype=np.float32),
    )
    x8 = _host_signs(x)
    in_maps = [
        {
            "xin8": x8[c * BPC : (c + 1) * BPC],
            "wts": wts,
            "sb": sbarr,
        }
        for c in range(NCORES)
    ]
    nc = get_nc()
    res = run_bass_kernel_spmd(nc, in_maps, list(range(NCORES)), trace=trace, **tkw)
    y = np.concatenate([r["yout"] for r in res.results], axis=0)
    return y.astype(np.float32, copy=False), res


def kernel(x, weight, gamma, beta, running_mean, running_var):
    y, _ = run(x, weight, gamma, beta, running_mean, running_var)
    return y


# revision 37
# speedup vs baseline: 1.0247x; 1.0244x over previous
"""Trainium2 Bass kernel for IR-Net style binarized 3x3 conv + BN + Hardtanh.

Reference computation:
  bw = sign(standardize(weight)) * sw   (sw = per-cout power-of-2 scale)
  ba = sign(x)
  y  = clip(conv3x3(ba, bw) * bn_scale + bn_bias, -1, 1)

Both matmul operands are exactly +-1, which is exactly representable in
fp8e4m3, so the conv runs as fp8 DoubleRow matmuls on the TensorEngine
with zero numerical error (fp32 PSUM accumulation of integers <= 2304).
All binarization is host-side prep: x ships as fp8 +-1 sign planes that
are already zero-padded and cin-chunk-interleaved, so activations DMA
straight into their SBUF matmul layout — no on-device binarize, border
memsets, or staging.  sw and the BN affine fold into one per-channel
scale/bias applied in the epilogue on VectorE.

Distribution: pure data parallel, 32 images -> 4 per NeuronCore, full
weights replicated, no collectives.

Layout: per-image zero-padded activation planes in SBUF, fp8, with the
two cin-128-chunks byte-interleaved as the DoubleRow k-subtile dim.
Rows are 57 wide (56 data + 1 shared zero column: col 0 is row r's left
pad AND row r-1's right pad), so each of the 9 conv taps is a contiguous
shifted window of the flattened plane and only 1 of every 57 output
columns is garbage.  The conv is 9 accumulated DoubleRow matmuls
([128,2,128] @ [128,2,456], K=256) per 8-row output tile.

Scheduling: dependency waits on DMA-written tiles coalesce to the
NEWEST DMA issued on that hardware ring at schedule time, so every
dma_start is placed in program order immediately before its first
consumer, split into just-in-time pieces (img0 in 3 row-bands, co=0
weights in 2 tap-groups).  A burst of dummy matmuls on a zeroed scratch
tile warms the PE HAM clock gate before the real stream starts.
"""

import numpy as np

import concourse.bass as bass
import concourse.bacc as bacc
import concourse.mybir as mybir
import concourse.tile as tile
from concourse.bass_utils import run_bass_kernel_spmd

B, CIN, COUT, H, W = 32, 256, 256, 56, 56
NCORES = 8
BPC = B // NCORES            # images per core
HP = H + 2                   # padded rows
RW = W + 1                   # row width: 56 data + 1 shared zero col
IMG = HP * RW                # 3306
GUARD = 64                   # front zero guard (shifted windows stay in bounds)
XT = 3376                    # GUARD + IMG + tail guard(6); %16==0 for DoubleRow
RB = 8                       # output rows per tile
NBLK = H // RB               # 7
NCI = CIN // 128             # 2 cin chunks = DoubleRow k-subtiles
NCO = COUT // 128            # 2 cout chunks
KTAPS = 9
BN_EPS = 1e-5

# img0 row-band split points (tile elem index): rows 0-10 / 11-26 / rest
S1 = GUARD + 11 * RW
S2 = GUARD + 27 * RW

F32 = mybir.dt.float32
FP8 = mybir.dt.float8e4
FP8NP = mybir.dt.np(FP8)

_CACHE: dict = {}


def _build_nc() -> bass.Bass:
    nc = bacc.Bacc("TRN2", target_bir_lowering=False, debug=False, num_devices=NCORES)
    xin8 = nc.declare_dram_parameter("xin8", [BPC, 128, XT * NCI], FP8, isOutput=False)
    wts = nc.declare_dram_parameter(
        "wts", [128, KTAPS * NCO * NCI * 128], FP8, isOutput=False
    )
    sb = nc.declare_dram_parameter("sb", [128, 2 * NCO], F32, isOutput=False)
    yout = nc.declare_dram_parameter("yout", [BPC, COUT, H, W], F32, isOutput=True)

    with tile.TileContext(nc) as tc:
        with (
            tc.tile_pool(name="const", bufs=1) as cpool,
            tc.tile_pool(name="psum", bufs=8, space=bass.MemorySpace.PSUM) as ppool,
            tc.tile_pool(name="ot", bufs=8) as otpool,
            tc.tile_pool(name="oc", bufs=12) as ocpool,
        ):
            # weights: [p, (co, k, j, m)]
            w_sb = cpool.tile([128, KTAPS * NCO * NCI * 128], FP8, tag="w")
            sb_sb = cpool.tile([128, 2 * NCO], F32, tag="sb")
            WTAP = NCI * 128          # 256 B per tap per partition
            WCO = KTAPS * WTAP        # one cout chunk
            w4 = w_sb.rearrange("p (co k j m) -> p k co j m", k=KTAPS, co=NCO, j=NCI)

            # Zero scratch for PE warmup operands (dedicated tile so warmup
            # reads never overlap anything written later).
            wz = cpool.tile([128, 512], FP8, tag="wz")
            nc.vector.memset(wz[:, 0:256], 0.0)
            nc.gpsimd.memset(wz[:, 256:512], 0.0)

            # Padded binarized activation planes, one tile per image;
            # entirely DMA-written (borders ship as zeros from the host).
            xp = {}
            for img in range(BPC):
                t = cpool.tile([128, XT, NCI], FP8, tag=f"xp{img}", name=f"xp{img}")
                xp[img] = t

            def ld_piece(img, a, b, eng):
                return eng.dma_start(
                    xp[img][:, a:b, :], xin8[img, :, a * NCI : b * NCI]
                )

            # Startup DMAs.  Any instruction that consumes DMA-written data
            # waits on the NEWEST DMA the scheduler issued before it (global
            # watermark), and the scheduler front-loads every ungated DMA.
            # So the ungated set is exactly what the first matmul needs —
            # img0 rows 0-10, scale/bias, co=0 weights (~455 KB) — and every
            # other transfer is semaphore-gated behind the first matmul.
            sc_chain = [nc.scalar.dma_start(sb_sb[:], sb[:])]
            sc_chain.append(nc.scalar.dma_start(w_sb[:, 0:WCO], wts[:, 0:WCO]))
            sq_chain = [ld_piece(0, 0, S1, nc.sync)]
            gq_chain = []

            # PE warmup: dummy DoubleRow matmuls on the zeroed scratch tile,
            # with operand access patterns identical in structure to the
            # real ones (the dual-fp8 LDWEIGHTS path is picky).  They only
            # depend on the scratch memset, so they start ~2us before the
            # first real matmul and flip the HAM clock gate to 8/8 by the
            # time the stream begins.
            # Normal-mode fp8, N=500: each MM covers ~275ns, so 14 of them
            # give ~3.9us of continuous PE busy — enough to flip the HAM
            # clock gate to 8/8 (needs ~3.4us) just before the real weights
            # land, so the real stream starts at full clock.  Once warm,
            # the sub-us handoff gap cannot re-throttle it.
            wm_ps = ppool.tile([128, 500], F32, tag="ps")
            wms = []
            for _ in range(14):
                wms.append(nc.tensor.matmul(
                    wm_ps[:],
                    wz[:, 0:128],
                    wz[:, 0:500],
                    start=True,
                    stop=True,
                ))

            def gate_dma(dma, trigger):
                # real semaphore gate on an early trigger (so the transfer
                # starts promptly) plus a schedule-order-only edge after the
                # first matmul (so mm0's global DMA watermark excludes it)
                tile.add_dep_helper(dma.ins, trigger.ins, sync=True,
                                    reason="JIT DMA trigger")
                tile.add_dep_helper(dma.ins, mm0.ins, sync=False,
                                    reason="keep out of mm0 watermark")

            mm0 = None
            for img in range(BPC):
                for co in range(NCO):
                    if img == 0 and co == 1:
                        # co=1 weights and the bulk image loads
                        wc1 = nc.scalar.dma_start(
                            w_sb[:, WCO : 2 * WCO], wts[:, WCO : 2 * WCO]
                        )
                        gate_dma(wc1, wms[8])
                        sc_chain.append(wc1)
                        for im2 in range(1, BPC):
                            dma = ld_piece(im2, 0, XT, nc.gpsimd)
                            gate_dma(dma, mm0)
                            gq_chain.append(dma)
                    s_ap = sb_sb[:, co : co + 1]
                    b_ap = sb_sb[:, NCO + co : NCO + co + 1]
                    # (start padded row, rows) per output tile; the final
                    # tiles of the kernel are split so the last epilogue +
                    # store chain after the last matmul is as short as
                    # possible, fanned out across otherwise-idle queues.
                    blocks = [(1 + b * RB, RB, nc.sync) for b in range(NBLK)]
                    if img == BPC - 1 and co == NCO - 1:
                        blocks = blocks[:-1] + [
                            (49, 4, nc.sync),
                            (53, 2, nc.gpsimd),
                            (55, 2, nc.scalar),
                        ]
                    for bi, (y0p, rb, oq) in enumerate(blocks):
                        if img == 0 and co == 0 and bi == 1:
                            # img0 rows 11-57 (two pieces), triggered off
                            # early warmup matmuls so they land before
                            # blocks 1 and 3 need them
                            for (a, b2), trig in (((S1, S2), wms[4]),
                                                  ((S2, XT), wms[6])):
                                dma = ld_piece(0, a, b2, nc.gpsimd)
                                gate_dma(dma, trig)
                                gq_chain.append(dma)
                        nt = rb * RW
                        ps = ppool.tile([128, nt], F32, tag="ps")
                        for k in range(KTAPS):
                            ky, kx = divmod(k, 3)
                            s0 = GUARD + (y0p + ky - 1) * RW + kx
                            rhs = xp[img][:, s0 : s0 + nt, :].rearrange(
                                "p x j -> p j x"
                            )
                            mm = nc.tensor.matmul(
                                ps[:],
                                w4[:, k, co],
                                rhs,
                                start=(k == 0),
                                stop=(k == KTAPS - 1),
                                perf_mode=mybir.MatmulPerfMode.DoubleRow,
                            )
                            if mm0 is None:
                                mm0 = mm

                        ot = otpool.tile([128, nt], F32, tag="ot")
                        nc.vector.tensor_scalar(
                            ot[:],
                            ps[:],
                            s_ap,
                            b_ap,
                            op0=mybir.AluOpType.mult,
                            op1=mybir.AluOpType.add,
                        )
                        # clip + compact away the garbage col per row, so
                        # both sides of the output DMA are fully contiguous
                        oc = ocpool.tile([128, rb * W], F32, tag="oc")
                        nc.vector.tensor_scalar(
                            oc[:],
                            ot.rearrange("p (r c) -> p r c", c=RW)[:, :, 0:W],
                            -1.0,
                            1.0,
                            op0=mybir.AluOpType.max,
                            op1=mybir.AluOpType.min,
                        )
                        oq.dma_start(
                            yout[img, co * 128 : (co + 1) * 128, y0p - 1 : y0p - 1 + rb, :],
                            oc[:],
                        )
            # pin issue order per ring (ring packet order = issue order)
            for ch in (sc_chain, sq_chain, gq_chain):
                for a, b in zip(ch, ch[1:]):
                    tile.add_dep_helper(
                        b.ins, a.ins, sync=False, reason="startup DMA issue order"
                    )
    nc.finalize()
    return nc


def get_nc() -> bass.Bass:
    if "nc" not in _CACHE:
        _CACHE["nc"] = _build_nc()
    return _CACHE["nc"]


def _host_prep(weight, gamma, beta, running_mean, running_var):
    """Binarize standardized weights, fold sw + BN into scale/bias."""
    wf = weight.reshape(COUT, -1).astype(np.float64)
    n = wf.shape[1]
    mean = wf.mean(axis=1, keepdims=True)
    d = wf - mean
    sgn = np.where(d >= 0, 1.0, -1.0)
    std = np.sqrt((d * d).sum(axis=1, keepdims=True) / (n - 1))
    bw = d / std
    sw = np.exp2(np.round(np.log2(np.abs(bw).mean(axis=1))))  # [COUT]
    inv = gamma.astype(np.float64) / np.sqrt(running_var.astype(np.float64) + BN_EPS)
    scale = (sw * inv).astype(np.float32)
    bias = (beta.astype(np.float64) - running_mean.astype(np.float64) * inv).astype(
        np.float32
    )

    # wts[p, (co, k, j, m)] = sgn[co*128+m, (j*128+p)*9 + k]
    w6 = sgn.reshape(NCO, 128, NCI, 128, KTAPS)  # [co, m, j, p, k]
    wts = (
        np.ascontiguousarray(np.transpose(w6, (3, 0, 4, 2, 1)))  # p co k j m
        .reshape(128, KTAPS * NCO * NCI * 128)
        .astype(FP8NP)
    )
    # sb[m, co] = scale chunk, sb[m, NCO+co] = bias chunk
    sbarr = np.concatenate(
        [scale.reshape(NCO, 128).T, bias.reshape(NCO, 128).T], axis=1
    ).astype(np.float32)
    sbarr = np.ascontiguousarray(sbarr)
    return wts, sbarr


def _host_signs(x):
    """fp8 +-1 sign planes, zero-padded 58x57 rows, cin-chunk interleaved.

    out[b, p, t, j] = fp8(sign(x[b, j*128+p, r-1, c-1])) at t = GUARD+r*57+c
    for the interior, 0 elsewhere (pads/guards), matching torch.sign
    (sign(0) = 0).
    """
    xv = x.reshape(B, NCI, 128, H, W)
    xs = ((xv < 0).astype(np.uint8) * 0x80) | ((xv != 0).astype(np.uint8) * 0x38)
    out = np.zeros((B, 128, XT, NCI), np.uint8)
    interior = out[:, :, GUARD : GUARD + IMG, :].reshape(B, 128, HP, RW, NCI)
    interior[:, :, 1 : H + 1, 1 : W + 1, :] = xs.transpose(0, 2, 3, 4, 1)
    return out.reshape(B, 128, XT * NCI).view(FP8NP)


def run(x, weight, gamma, beta, running_mean, running_var, trace=False, **tkw):
    x = np.asarray(x, dtype=np.float32)
    wts, sbarr = _host_prep(
        np.asarray(weight, dtype=np.float32),
        np.asarray(gamma, dtype=np.float32),
        np.asarray(beta, dtype=np.float32),
        np.asarray(running_mean, dtype=np.float32),
        np.asarray(running_var, dtype=np.float32),
    )
    x8 = _host_signs(x)
    in_maps = [
        {
            "xin8": x8[c * BPC : (c + 1) * BPC],
            "wts": wts,
            "sb": sbarr,
        }
        for c in range(NCORES)
    ]
    nc = get_nc()
    res = run_bass_kernel_spmd(nc, in_maps, list(range(NCORES)), trace=trace, **tkw)
    y = np.concatenate([r["yout"] for r in res.results], axis=0)
    return y.astype(np.float32, copy=False), res


def kernel(x, weight, gamma, beta, running_mean, running_var):
    y, _ = run(x, weight, gamma, beta, running_mean, running_var)
    return y
